# revision 24
# baseline (speedup 1.0000x reference)
"""Trainium2 Bass kernel for nn_MoE_12317966205421 (capsule-expert MoE).

Strategy: spatial sharding over 8 cores (12 image rows each + 1-row halo).
Every core runs the same program: for each sample b, for each expert e that
any of the 4 gates selected for b (union of top-2 sets), compute
  u = conv3x3(x_b, Wc_e) + bc_e ; v = relu(u) ; s = squash(v) ; p = Wp_e^T s
and accumulate w[g,e,b] * p into 4 per-gate outputs. Gating (softmax, top-k,
renormalized weights, cv^2 loss) is computed on host (tiny: B=8, E=8).

Conv = 9 shifted fp32r matmuls accumulating in PSUM over a zero-padded
98-wide row layout. Squash channel-sum via an all-ones fp32r matmul which
also broadcasts the per-pixel sum to all 128 partitions.

The per-pair work is software-pipelined across 3 stages so the PE stream
(conv of pair i, squash-sum of pair i-1, projection of pair i-2) never
waits on the ACT/DVE/GPSIMD squash chain.
"""
import sys

import numpy as np

if '/opt/trn_rl_repo' in sys.path:
    sys.path.remove('/opt/trn_rl_repo')

import concourse.bacc as bacc
import concourse.tile as tile
from concourse import bass_utils, mybir

F32 = mybir.dt.float32
F32R = mybir.dt.float32r
F16 = mybir.dt.float16
AF = mybir.ActivationFunctionType
OP = mybir.AluOpType

B, C, H, W, E, NUM_GATES, TOP = 8, 128, 96, 96, 8, 4, 2
NCORES = 8
R = H // NCORES          # 12 output rows per core
WP = W + 2               # 98 padded width
NIN = 14 * WP            # input rows incl. halo, flattened
XLEN = NIN + 2           # +1 pad element at each end
NCH, NCOL = 3, 392       # 3 chunks of 4 rows x 98
NPIX = NCH * NCOL        # 1176


def _softmax(x, axis=-1):
    m = np.max(x, axis=axis, keepdims=True)
    e = np.exp(x - m)
    return e / np.sum(e, axis=axis, keepdims=True)


def _host_gates(x, G):
    """Returns w_dense [NUM_GATES, E, B] fp32 and total_loss fp32 scalar."""
    x_gap = x.reshape(B, C, H * W).mean(axis=2, dtype=np.float64).astype(np.float32)
    w_dense = np.zeros((NUM_GATES, E, B), np.float32)
    losses = []
    for g in range(NUM_GATES):
        logits = x_gap @ G[g]                       # [B, E]
        probs = _softmax(logits.astype(np.float32), axis=1)
        ti = np.argsort(-probs, axis=1, kind="stable")[:, :TOP]   # [B, TOP]
        tv = np.take_along_axis(probs, ti, axis=1)
        tw = _softmax(tv, axis=1)
        for b in range(B):
            for k in range(TOP):
                w_dense[g, ti[b, k], b] += tw[b, k]
        v = probs.sum(axis=0).astype(np.float64)    # [E]
        loss = v.var(ddof=1) / (v.mean() ** 2 + 1e-10)
        losses.append(loss)
    total_loss = np.float32(np.mean(losses))
    return w_dense, total_loss


def _build_structure(w_dense):
    """Per-b expert lists and per-(e,b) gate hits with first-hit flags."""
    pairs = []   # list of (b, e, [(g, w, is_first_for_gate)])
    for b in range(B):
        experts = [e for e in range(E) if np.any(w_dense[:, e, b] != 0.0)]
        seen_g = set()
        for e in experts:
            hits = []
            for g in range(NUM_GATES):
                wv = float(w_dense[g, e, b])
                if wv != 0.0:
                    hits.append((g, wv, g not in seen_g))
                    seen_g.add(g)
            pairs.append((b, e, hits))
    return pairs


def _build_program(pairs):
    nc = bacc.Bacc("TRN2", target_bir_lowering=False, debug=False,
                   num_devices=NCORES)
    xs = nc.dram_tensor("xs", [C, B, XLEN], F16, kind="ExternalInput")
    wc = nc.dram_tensor("wc", [C, E, 9, C], F16, kind="ExternalInput")
    wpr = nc.dram_tensor("wpr", [C, E, C], F16, kind="ExternalInput")
    ones = nc.dram_tensor("ones", [C, C], F16, kind="ExternalInput")
    bcb = nc.dram_tensor("bcb", [C, E], F32, kind="ExternalInput")
    bout = nc.dram_tensor("bout", [C, NUM_GATES, B], F32, kind="ExternalInput")
    eps = nc.dram_tensor("eps", [C, 1], F32, kind="ExternalInput")
    out = nc.dram_tensor("out", [NUM_GATES, B, C, NPIX], F32,
                         kind="ExternalOutput")

    n = len(pairs)
    last_pair_of_b = {}
    for i, (b_, _, _) in enumerate(pairs):
        last_pair_of_b[b_] = i

    with tile.TileContext(nc) as tc:
        with tc.tile_pool(name="const", bufs=1) as cpool, \
             tc.tile_pool(name="big", bufs=3) as bigp, \
             tc.tile_pool(name="sp", bufs=4) as spool, \
             tc.tile_pool(name="mid", bufs=3) as midp, \
             tc.tile_pool(name="mid1", bufs=2) as midp1, \
             tc.tile_pool(name="acc", bufs=3) as accp, \
             tc.tile_pool(name="pconv", bufs=1, space="PSUM") as pconv, \
             tc.tile_pool(name="psnb", bufs=1, space="PSUM") as psnb, \
             tc.tile_pool(name="pproj", bufs=2, space="PSUM") as pproj:

            txs = cpool.tile([C, B, XLEN], F16)
            twc = cpool.tile([C, E, 9, C], F16)
            twp = cpool.tile([C, E, C], F16)
            tones = cpool.tile([C, C], F16)
            tbc = cpool.tile([C, E], F32)
            tbout = cpool.tile([C, NUM_GATES, B], F32)
            teps = cpool.tile([C, 1], F32)
            nc.sync.dma_start(twc[:], wc.ap())
            nc.sync.dma_start(txs[:], xs.ap())
            nc.sync.dma_start(twp[:], wpr.ap())
            nc.sync.dma_start(tones[:], ones.ap())
            nc.sync.dma_start(tbc[:], bcb.ap())
            nc.sync.dma_start(tbout[:], bout.ap())
            nc.sync.dma_start(teps[:], eps.ap())

            ctx = {}          # pair index -> dict of tiles
            accs = {}         # gate -> tile, for the b currently in tail
            last_on = {}      # engine chain: key -> last BassInstruction

            def chain(key, bi):
                prev = last_on.get(key)
                if prev is not None:
                    tile.add_dep_helper(bi.ins, prev.ins, False,
                                        f"{key} order")
                last_on[key] = bi

            def conv_taps(i, taps):
                b_, e, _ = pairs[i]
                st = ctx[i]
                for t in taps:
                    off = (t // 3) * WP + (t % 3) - (WP + 1)
                    for c in range(NCH):
                        s0 = 99 + c * NCOL + off
                        chain("pe", nc.tensor.matmul(
                            st["cps"][c][:, 0:NCOL], twc[:, e, t, :],
                            txs[:, b_, s0:s0 + NCOL],
                            start=(t == 0), stop=(t == 8),
                            skip_group_check=True))

            def snb_block(j):
                st = ctx[j]
                sb = psnb.tile([C, NCH, 512], F32, tag="snb", name="snb")
                st["snb"] = sb
                for c in range(NCH):
                    chain("pe", nc.tensor.matmul(
                        sb[:, c, 0:NCOL], tones[:],
                        st["s2"][:, c * NCOL:(c + 1) * NCOL],
                        start=True, stop=True, skip_group_check=True))


            def act_relu(i):
                b_, e, _ = pairs[i]
                st = ctx[i]
                st["v"] = bigp.tile([C, NPIX], F16, tag="v", name="v")
                for c in range(NCH):
                    cs = slice(c * NCOL, (c + 1) * NCOL)
                    chain("act", nc.scalar.activation(
                        st["v"][:, cs], st["cps"][c][:, 0:NCOL],
                        AF.Relu, bias=tbc[:, e:e + 1]))

            def act_square(i):
                st = ctx[i]
                st["s2"] = bigp.tile([C, NPIX], F16, tag="s2", name="s2")
                chain("act", nc.scalar.activation(st["s2"][:], st["v"][:],
                                                  AF.Square))

            def act_sqrt(j):
                st = ctx[j]
                chain("act", nc.scalar.activation(
                    st["q"][:].rearrange("p (c n) -> p c n", c=NCH),
                    st["snb"][:, :, 0:NCOL], AF.Sqrt, bias=teps[:, 0:1]))
                chain("act", nc.scalar.activation(
                    st["sn"][:].rearrange("p (c n) -> p c n", c=NCH),
                    st["snb"][:, :, 0:NCOL], AF.Copy))

            def chain_dr(j, c):
                st = ctx[j]
                cs = slice(c * NCOL, (c + 1) * NCOL)
                chain("dve", nc.vector.scalar_tensor_tensor(
                    st["d"][:, cs], st["sn"][:, cs], 1.0,
                    st["q"][:, cs], OP.add, OP.mult))
                chain("dve", nc.vector.reciprocal_approx_fast(
                    st["r"][:, cs], st["d"][:, cs]))

            def chain_fs(j):
                st = ctx[j]
                f = midp1.tile([C, NPIX], F32, tag="f", name="f")
                chain("gps", nc.gpsimd.tensor_tensor(f[:], st["sn"][:],
                                                     st["r"][:], OP.mult))
                sq = spool.tile([C, NPIX], F16, tag="s", name="s")
                chain("gps", nc.gpsimd.tensor_tensor(sq[:], st["v"][:], f[:],
                                                     OP.mult))
                st["s"] = sq

            def proj_chunk(k, c):
                b_, e, hits = pairs[k]
                st = ctx[k]
                pp = pproj.tile([C, 512], F32, tag="proj", name="proj")
                st["proj"][c] = pp
                cs = slice(c * NCOL, (c + 1) * NCOL)
                chain("pe", nc.tensor.matmul(
                    pp[:, 0:NCOL], twp[:, e, :], st["s"][:, cs],
                    start=True, stop=True, skip_group_check=True))

            def hits_chunk(k, c):
                b_, e, hits = pairs[k]
                st = ctx[k]
                pp = st["proj"][c]
                cs = slice(c * NCOL, (c + 1) * NCOL)
                for (g, wv, first) in hits:
                    if first:
                        chain("dve", nc.vector.tensor_scalar(
                            accs[g][:, cs], pp[:, 0:NCOL], wv,
                            tbout[:, g, b_:b_ + 1], OP.mult, OP.add))
                    else:
                        chain("dve", nc.vector.scalar_tensor_tensor(
                            accs[g][:, cs], pp[:, 0:NCOL], wv,
                            accs[g][:, cs], OP.mult, OP.add))

            for i in range(n + 3):
                if i < n:
                    ctx[i] = {"cps": [pconv.tile([C, 512], F32,
                                                 tag=f"cps{c}", name="cps")
                                      for c in range(NCH)],
                              "proj": {}}
                if 0 <= i - 1 < n:
                    st1 = ctx[i - 1]
                    st1["q"] = midp.tile([C, NPIX], F32, tag="q", name="q")
                    st1["sn"] = midp.tile([C, NPIX], F32, tag="sn", name="sn")
                    st1["d"] = midp1.tile([C, NPIX], F32, tag="d", name="d")
                    st1["r"] = midp1.tile([C, NPIX], F32, tag="r", name="r")
                if i - 3 >= 0:
                    bk = pairs[i - 3][0]
                    if i - 3 == 0 or pairs[i - 4][0] != bk:
                        for g in range(NUM_GATES):
                            accs[g] = accp.tile([C, NPIX], F32,
                                                tag=f"acc{g}", name=f"acc{g}")

                prev = 0 <= i - 1 < n
                p3 = i - 3 >= 0

                # PE: proj chunks of (i-3) spread through the step so the
                # DVE hit-reads can free each proj bank before its slot is
                # reused; snb(i-1) mid-step (square(i-1) lands early here).
                if i < n:
                    if p3:
                        proj_chunk(i - 3, 0)
                    conv_taps(i, [0, 1])
                    if p3:
                        proj_chunk(i - 3, 1)
                    if prev:
                        snb_block(i - 1)
                    conv_taps(i, [2, 3])
                    if p3:
                        proj_chunk(i - 3, 2)
                    conv_taps(i, [4, 5, 6, 7, 8])
                else:
                    if p3:
                        for c in range(NCH):
                            proj_chunk(i - 3, c)
                    if prev:
                        snb_block(i - 1)

                # ACT (in-order): sqrt(i-1), then relu/square(i)
                if prev:
                    act_sqrt(i - 1)
                if i < n:
                    act_relu(i)
                    act_square(i)

                # DVE: gate accumulation of proj(i-3), then squash chain
                if p3:
                    for c in range(NCH):
                        hits_chunk(i - 3, c)
                if prev:
                    for c in range(NCH):
                        chain_dr(i - 1, c)

                # GPSIMD
                if prev:
                    chain_fs(i - 1)

                # flush finished sample
                if p3:
                    bk = pairs[i - 3][0]
                    if last_pair_of_b[bk] == i - 3:
                        for g in range(NUM_GATES):
                            chain("outd", nc.sync.dma_start(out.ap()[g, bk],
                                                            accs[g][:]))

                if i - 3 >= 0:
                    ctx.pop(i - 3)
    nc.compile()
    return nc


def _prep_inputs(x, Wc, bc, Wp, w_dense, bp):
    """Host-side tensor prep. Returns (shared dict, per-core xs list)."""
    f32 = np.float32
    wc_prep = np.ascontiguousarray(
        np.transpose(Wc.reshape(E, C, C, 9), (2, 0, 3, 1)).astype(np.float16))
    wp_prep = np.ascontiguousarray(np.transpose(Wp, (2, 0, 1)).astype(np.float16))
    ones_prep = np.ones((C, C), np.float16)
    bc_prep = np.ascontiguousarray(bc.T.astype(f32))
    bout_prep = np.ascontiguousarray(
        np.einsum("geb,ec->cgb", w_dense, bp).astype(f32))

    x_pad = np.zeros((B, C, H + 2, WP), f32)
    x_pad[:, :, 1:97, 1:97] = x
    xs_cores = []
    for k in range(NCORES):
        sl = x_pad[:, :, 12 * k:12 * k + 14, :].reshape(B, C, NIN)
        xsk = np.zeros((C, B, XLEN), np.float16)
        xsk[:, :, 1:1 + NIN] = np.transpose(sl, (1, 0, 2)).astype(np.float16)
        xs_cores.append(xsk)
    shared = {"wc": wc_prep, "wpr": wp_prep, "ones": ones_prep,
              "bcb": bc_prep, "bout": bout_prep,
              "eps": np.full((C, 1), 1e-8, f32)}
    return shared, xs_cores


_NC_CACHE = {}


def _get_program(pairs_key, pairs):
    if pairs_key not in _NC_CACHE:
        _NC_CACHE[pairs_key] = _build_program(pairs)
    return _NC_CACHE[pairs_key]


def kernel(x, G, Wc, bc, Wp, bp, _trace=False):
    x = np.asarray(x, np.float32)
    G = np.asarray(G, np.float32)
    Wc = np.asarray(Wc, np.float32)
    bc = np.asarray(bc, np.float32)
    Wp = np.asarray(Wp, np.float32)
    bp = np.asarray(bp, np.float32)

    w_dense, total_loss = _host_gates(x, G)
    pairs = _build_structure(w_dense)
    pairs_key = tuple((b, e, tuple((g, first) for g, _, first in h))
                      for b, e, h in pairs) + tuple(
                          np.round(w_dense, 7).ravel().tolist())
    nc = _get_program(pairs_key, pairs)

    shared, xs_cores = _prep_inputs(x, Wc, bc, Wp, w_dense, bp)
    in_maps = [{**shared, "xs": xs_cores[k]} for k in range(NCORES)]
    res = bass_utils.run_bass_kernel_spmd(
        nc, in_maps, core_ids=list(range(NCORES)), trace=_trace)

    ys = [np.zeros((B, C, H, W), np.float32) for _ in range(NUM_GATES)]
    for k in range(NCORES):
        o = res.results[k]["out"]    # [4, B, C, NPIX]
        ov = o.reshape(NUM_GATES, B, C, R, WP)[:, :, :, :, 1:97]
        for g in range(NUM_GATES):
            ys[g][:, :, 12 * k:12 * k + 12, :] = ov[g]
    if _trace:
        kernel.last_result = res
    return (*ys, total_loss)


# revision 25
# speedup vs baseline: 1.2198x; 1.2198x over previous
"""Trainium2 Bass kernel for nn_MoE_12317966205421 (capsule-expert MoE).

Strategy: spatial sharding over 8 cores (12 image rows each + 1-row halo).
Every core runs the same program: for each sample b, for each expert e that
any of the 4 gates selected for b (union of top-2 sets), compute
  u = conv3x3(x_b, Wc_e) + bc_e ; v = relu(u) ; s = squash(v) ; p = Wp_e^T s
and accumulate w[g,e,b] * p into 4 per-gate outputs. Gating (softmax, top-k,
renormalized weights, cv^2 loss) is computed on host (tiny: B=8, E=8).

Conv = 9 shifted fp32r matmuls accumulating in PSUM over a zero-padded
98-wide row layout. Squash channel-sum via an all-ones fp32r matmul which
also broadcasts the per-pixel sum to all 128 partitions.

The per-pair work is software-pipelined across 3 stages so the PE stream
(conv of pair i, squash-sum of pair i-1, projection of pair i-2) never
waits on the ACT/DVE/GPSIMD squash chain.
"""
import sys

import numpy as np

if '/opt/trn_rl_repo' in sys.path:
    sys.path.remove('/opt/trn_rl_repo')

import concourse.bacc as bacc
import concourse.tile as tile
from concourse import bass_utils, mybir

F32 = mybir.dt.float32
F32R = mybir.dt.float32r
F16 = mybir.dt.float16
AF = mybir.ActivationFunctionType
OP = mybir.AluOpType

B, C, H, W, E, NUM_GATES, TOP = 8, 128, 96, 96, 8, 4, 2
NCORES = 8
R = H // NCORES          # 12 output rows per core
WP = W + 2               # 98 padded width
NIN = 14 * WP            # input rows incl. halo, flattened
XLEN = NIN + 2           # +1 pad element at each end
NCH, NCOL = 3, 392       # 3 chunks of 4 rows x 98
NPIX = NCH * NCOL        # 1176


def _softmax(x, axis=-1):
    m = np.max(x, axis=axis, keepdims=True)
    e = np.exp(x - m)
    return e / np.sum(e, axis=axis, keepdims=True)


def _host_gates(x, G):
    """Returns w_dense [NUM_GATES, E, B] fp32 and total_loss fp32 scalar."""
    x_gap = x.reshape(B, C, H * W).mean(axis=2, dtype=np.float64).astype(np.float32)
    w_dense = np.zeros((NUM_GATES, E, B), np.float32)
    losses = []
    for g in range(NUM_GATES):
        logits = x_gap @ G[g]                       # [B, E]
        probs = _softmax(logits.astype(np.float32), axis=1)
        ti = np.argsort(-probs, axis=1, kind="stable")[:, :TOP]   # [B, TOP]
        tv = np.take_along_axis(probs, ti, axis=1)
        tw = _softmax(tv, axis=1)
        for b in range(B):
            for k in range(TOP):
                w_dense[g, ti[b, k], b] += tw[b, k]
        v = probs.sum(axis=0).astype(np.float64)    # [E]
        loss = v.var(ddof=1) / (v.mean() ** 2 + 1e-10)
        losses.append(loss)
    total_loss = np.float32(np.mean(losses))
    return w_dense, total_loss


def _build_structure(w_dense):
    """Per-b expert lists and per-(e,b) gate hits with first-hit flags."""
    pairs = []   # list of (b, e, [(g, w, is_first_for_gate)])
    for b in range(B):
        experts = [e for e in range(E) if np.any(w_dense[:, e, b] != 0.0)]
        seen_g = set()
        for e in experts:
            hits = []
            for g in range(NUM_GATES):
                wv = float(w_dense[g, e, b])
                if wv != 0.0:
                    hits.append((g, wv, g not in seen_g))
                    seen_g.add(g)
            pairs.append((b, e, hits))
    return pairs


def _build_program(pairs):
    nc = bacc.Bacc("TRN2", target_bir_lowering=False, debug=False,
                   num_devices=NCORES)
    xs = nc.dram_tensor("xs", [C, B, XLEN], F16, kind="ExternalInput")
    wc = nc.dram_tensor("wc", [C, E, 9, C], F16, kind="ExternalInput")
    wpr = nc.dram_tensor("wpr", [C, E, C], F16, kind="ExternalInput")
    ones = nc.dram_tensor("ones", [C, C], F16, kind="ExternalInput")
    bcb = nc.dram_tensor("bcb", [C, E], F32, kind="ExternalInput")
    bout = nc.dram_tensor("bout", [C, NUM_GATES, B], F32, kind="ExternalInput")
    eps = nc.dram_tensor("eps", [C, 1], F32, kind="ExternalInput")
    out = nc.dram_tensor("out", [NUM_GATES, B, C, NPIX], F32,
                         kind="ExternalOutput")

    n = len(pairs)
    last_pair_of_b = {}
    for i, (b_, _, _) in enumerate(pairs):
        last_pair_of_b[b_] = i

    with tile.TileContext(nc) as tc:
        with tc.tile_pool(name="const", bufs=1) as cpool, \
             tc.tile_pool(name="big", bufs=3) as bigp, \
             tc.tile_pool(name="sp", bufs=5) as spool, \
             tc.tile_pool(name="mid", bufs=3) as midp, \
             tc.tile_pool(name="mid1", bufs=2) as midp1, \
             tc.tile_pool(name="acc", bufs=3) as accp, \
             tc.tile_pool(name="pconv", bufs=1, space="PSUM") as pconv, \
             tc.tile_pool(name="psnb", bufs=1, space="PSUM") as psnb, \
             tc.tile_pool(name="pproj", bufs=2, space="PSUM") as pproj:

            txs = cpool.tile([C, B, XLEN], F16)
            twc = cpool.tile([C, E, 9, C], F16)
            twp = cpool.tile([C, E, C], F16)
            tones = cpool.tile([C, C], F16)
            tbc = cpool.tile([C, E], F32)
            tbout = cpool.tile([C, NUM_GATES, B], F32)
            teps = cpool.tile([C, 1], F32)
            nc.sync.dma_start(twc[:], wc.ap())
            nc.sync.dma_start(txs[:], xs.ap())
            nc.sync.dma_start(twp[:], wpr.ap())
            nc.sync.dma_start(tones[:], ones.ap())
            nc.sync.dma_start(tbc[:], bcb.ap())
            nc.sync.dma_start(tbout[:], bout.ap())
            nc.sync.dma_start(teps[:], eps.ap())

            ctx = {}          # pair index -> dict of tiles
            accs = {}         # gate -> tile, for the b currently in tail
            last_on = {}      # engine chain: key -> last BassInstruction

            def chain(key, bi):
                prev = last_on.get(key)
                if prev is not None:
                    tile.add_dep_helper(bi.ins, prev.ins, False,
                                        f"{key} order")
                last_on[key] = bi

            def conv_taps(i, taps):
                b_, e, _ = pairs[i]
                st = ctx[i]
                for t in taps:
                    off = (t // 3) * WP + (t % 3) - (WP + 1)
                    for c in range(NCH):
                        s0 = 99 + c * NCOL + off
                        chain("pe", nc.tensor.matmul(
                            st["cps"][c][:, 0:NCOL], twc[:, e, t, :],
                            txs[:, b_, s0:s0 + NCOL],
                            start=(t == 0), stop=(t == 8),
                            skip_group_check=True))

            def snb_block(j):
                st = ctx[j]
                sb = psnb.tile([C, NCH, 512], F32, tag="snb", name="snb")
                st["snb"] = sb
                for c in range(NCH):
                    chain("pe", nc.tensor.matmul(
                        sb[:, c, 0:NCOL], tones[:],
                        st["s2"][:, c * NCOL:(c + 1) * NCOL],
                        start=True, stop=True, skip_group_check=True))


            def act_relu(i):
                b_, e, _ = pairs[i]
                st = ctx[i]
                st["v"] = bigp.tile([C, NPIX], F16, tag="v", name="v")
                for c in range(NCH):
                    cs = slice(c * NCOL, (c + 1) * NCOL)
                    chain("act", nc.scalar.activation(
                        st["v"][:, cs], st["cps"][c][:, 0:NCOL],
                        AF.Relu, bias=tbc[:, e:e + 1]))

            def act_square(i):
                st = ctx[i]
                st["s2"] = bigp.tile([C, NPIX], F16, tag="s2", name="s2")
                chain("act", nc.scalar.activation(st["s2"][:], st["v"][:],
                                                  AF.Square))

            def act_sqrt(j):
                st = ctx[j]
                chain("act", nc.scalar.activation(
                    st["q"][:].rearrange("p (c n) -> p c n", c=NCH),
                    st["snb"][:, :, 0:NCOL], AF.Sqrt, bias=teps[:, 0:1]))
                chain("act", nc.scalar.activation(
                    st["sn"][:].rearrange("p (c n) -> p c n", c=NCH),
                    st["snb"][:, :, 0:NCOL], AF.Copy))

            def chain_dr(j, c):
                st = ctx[j]
                cs = slice(c * NCOL, (c + 1) * NCOL)
                chain("dve", nc.vector.scalar_tensor_tensor(
                    st["d"][:, cs], st["sn"][:, cs], 1.0,
                    st["q"][:, cs], OP.add, OP.mult))
                chain("dve", nc.vector.reciprocal_approx_fast(
                    st["r"][:, cs], st["d"][:, cs]))

            def chain_fs(j):
                st = ctx[j]
                f = midp1.tile([C, NPIX], F32, tag="f", name="f")
                chain("gps", nc.gpsimd.tensor_tensor(f[:], st["sn"][:],
                                                     st["r"][:], OP.mult))
                sq = spool.tile([C, NPIX], F16, tag="s", name="s")
                chain("gps", nc.gpsimd.tensor_tensor(sq[:], st["v"][:], f[:],
                                                     OP.mult))
                st["s"] = sq

            def proj_chunk(k, c):
                b_, e, hits = pairs[k]
                st = ctx[k]
                pp = pproj.tile([C, 512], F32, tag="proj", name="proj")
                st["proj"][c] = pp
                cs = slice(c * NCOL, (c + 1) * NCOL)
                chain("pe", nc.tensor.matmul(
                    pp[:, 0:NCOL], twp[:, e, :], st["s"][:, cs],
                    start=True, stop=True, skip_group_check=True))

            def hits_chunk(k, c):
                b_, e, hits = pairs[k]
                st = ctx[k]
                pp = st["proj"][c]
                cs = slice(c * NCOL, (c + 1) * NCOL)
                for (g, wv, first) in hits:
                    if first:
                        chain("dve", nc.vector.tensor_scalar(
                            accs[g][:, cs], pp[:, 0:NCOL], wv,
                            tbout[:, g, b_:b_ + 1], OP.mult, OP.add))
                    else:
                        chain("dve", nc.vector.scalar_tensor_tensor(
                            accs[g][:, cs], pp[:, 0:NCOL], wv,
                            accs[g][:, cs], OP.mult, OP.add))

            for i in range(n + 4):
                if i < n:
                    ctx[i] = {"cps": [pconv.tile([C, 512], F32,
                                                 tag=f"cps{c}", name="cps")
                                      for c in range(NCH)],
                              "proj": {}}
                if 0 <= i - 1 < n:
                    st1 = ctx[i - 1]
                    st1["q"] = midp.tile([C, NPIX], F32, tag="q", name="q")
                    st1["sn"] = midp.tile([C, NPIX], F32, tag="sn", name="sn")
                    st1["d"] = midp1.tile([C, NPIX], F32, tag="d", name="d")
                    st1["r"] = midp1.tile([C, NPIX], F32, tag="r", name="r")
                if i - 4 >= 0:
                    bk = pairs[i - 4][0]
                    if i - 4 == 0 or pairs[i - 5][0] != bk:
                        for g in range(NUM_GATES):
                            accs[g] = accp.tile([C, NPIX], F32,
                                                tag=f"acc{g}", name=f"acc{g}")

                prev = 0 <= i - 1 < n
                p3 = i - 4 >= 0

                # PE: proj chunks of (i-3) spread through the step so the
                # DVE hit-reads can free each proj bank before its slot is
                # reused; snb(i-1) mid-step (square(i-1) lands early here).
                if i < n:
                    if p3:
                        proj_chunk(i - 4, 0)
                    conv_taps(i, [0, 1])
                    if p3:
                        proj_chunk(i - 4, 1)
                    if prev:
                        snb_block(i - 1)
                    conv_taps(i, [2, 3])
                    if p3:
                        proj_chunk(i - 4, 2)
                    conv_taps(i, [4, 5, 6, 7, 8])
                else:
                    if p3:
                        for c in range(NCH):
                            proj_chunk(i - 4, c)
                    if prev:
                        snb_block(i - 1)

                # ACT (in-order): sqrt(i-1), then relu/square(i)
                if prev:
                    act_sqrt(i - 1)
                if i < n:
                    act_relu(i)
                    act_square(i)

                # DVE: gate accumulation of proj(i-3), then squash chain
                if p3:
                    for c in range(NCH):
                        hits_chunk(i - 4, c)
                if prev:
                    for c in range(NCH):
                        chain_dr(i - 1, c)

                # GPSIMD
                if prev:
                    chain_fs(i - 1)

                # flush finished sample
                if p3:
                    bk = pairs[i - 4][0]
                    if last_pair_of_b[bk] == i - 4:
                        for g in range(NUM_GATES):
                            chain("outd", nc.sync.dma_start(out.ap()[g, bk],
                                                            accs[g][:]))

                if i - 4 >= 0:
                    ctx.pop(i - 4)
    nc.compile()
    return nc


def _prep_inputs(x, Wc, bc, Wp, w_dense, bp):
    """Host-side tensor prep. Returns (shared dict, per-core xs list)."""
    f32 = np.float32
    wc_prep = np.ascontiguousarray(
        np.transpose(Wc.reshape(E, C, C, 9), (2, 0, 3, 1)).astype(np.float16))
    wp_prep = np.ascontiguousarray(np.transpose(Wp, (2, 0, 1)).astype(np.float16))
    ones_prep = np.ones((C, C), np.float16)
    bc_prep = np.ascontiguousarray(bc.T.astype(f32))
    bout_prep = np.ascontiguousarray(
        np.einsum("geb,ec->cgb", w_dense, bp).astype(f32))

    x_pad = np.zeros((B, C, H + 2, WP), f32)
    x_pad[:, :, 1:97, 1:97] = x
    xs_cores = []
    for k in range(NCORES):
        sl = x_pad[:, :, 12 * k:12 * k + 14, :].reshape(B, C, NIN)
        xsk = np.zeros((C, B, XLEN), np.float16)
        xsk[:, :, 1:1 + NIN] = np.transpose(sl, (1, 0, 2)).astype(np.float16)
        xs_cores.append(xsk)
    shared = {"wc": wc_prep, "wpr": wp_prep, "ones": ones_prep,
              "bcb": bc_prep, "bout": bout_prep,
              "eps": np.full((C, 1), 1e-8, f32)}
    return shared, xs_cores


_NC_CACHE = {}


def _get_program(pairs_key, pairs):
    if pairs_key not in _NC_CACHE:
        _NC_CACHE[pairs_key] = _build_program(pairs)
    return _NC_CACHE[pairs_key]


def kernel(x, G, Wc, bc, Wp, bp, _trace=False):
    x = np.asarray(x, np.float32)
    G = np.asarray(G, np.float32)
    Wc = np.asarray(Wc, np.float32)
    bc = np.asarray(bc, np.float32)
    Wp = np.asarray(Wp, np.float32)
    bp = np.asarray(bp, np.float32)

    w_dense, total_loss = _host_gates(x, G)
    pairs = _build_structure(w_dense)
    pairs_key = tuple((b, e, tuple((g, first) for g, _, first in h))
                      for b, e, h in pairs) + tuple(
                          np.round(w_dense, 7).ravel().tolist())
    nc = _get_program(pairs_key, pairs)

    shared, xs_cores = _prep_inputs(x, Wc, bc, Wp, w_dense, bp)
    in_maps = [{**shared, "xs": xs_cores[k]} for k in range(NCORES)]
    res = bass_utils.run_bass_kernel_spmd(
        nc, in_maps, core_ids=list(range(NCORES)), trace=_trace)

    ys = [np.zeros((B, C, H, W), np.float32) for _ in range(NUM_GATES)]
    for k in range(NCORES):
        o = res.results[k]["out"]    # [4, B, C, NPIX]
        ov = o.reshape(NUM_GATES, B, C, R, WP)[:, :, :, :, 1:97]
        for g in range(NUM_GATES):
            ys[g][:, :, 12 * k:12 * k + 12, :] = ov[g]
    if _trace:
        kernel.last_result = res
    return (*ys, total_loss)
